# revision 1
# baseline (speedup 1.0000x reference)
import numpy as np

N_NODES = 50000
OUT_FEATS = 128


def kernel(h, W, b, src, dst):
    """GCN layer: relu(segment_sum((h @ W)[src], dst) + b).

    Full-input / full-output contract. Computes the dense transform once
    on the node dim, gathers per-edge messages, scatter-adds into dst
    rows, then applies bias + relu.
    """
    h = np.asarray(h, dtype=np.float32)
    W = np.asarray(W, dtype=np.float32)
    b = np.asarray(b, dtype=np.float32)
    src = np.asarray(src).astype(np.int64)
    dst = np.asarray(dst).astype(np.int64)

    hw = h @ W  # [N, out]

    agg = np.zeros((N_NODES, OUT_FEATS), dtype=np.float32)
    # sort edges by dst so the scatter-add becomes a segmented reduce
    order = np.argsort(dst, kind="stable")
    dst_s = dst[order]
    msgs = hw[src[order]]  # [E, out]
    np.add.at(agg, dst_s, msgs)

    out = agg + b[None, :]
    np.maximum(out, 0.0, out=out)
    return out



# revision 2
# speedup vs baseline: 1420.0661x; 1420.0661x over previous
"""GCN layer on 8 TRN2 NeuronCores (Bass/Tile kernel).

out = relu(segment_sum((h @ W)[src], dst) + b)

Self-contained: hardcodes the problem shapes (N=50000, IN=256, OUT=128,
E=800000) and the sharding strategy.

Strategy (dst-sharded, 8 cores):
  Phase 1: each core computes hw = h_shard @ W in bf16 on the PE, writes its
           shard to internal DRAM, AllGather -> full hw table on every core.
  Phase 2: edges partitioned by dst owner; per core, edges grouped by
           (dst window of 128 rows, src half table) and padded to 128-edge
           chunks.  dma_gather (SWDGE) pulls per-edge message rows hw[src]
           from DRAM into SBUF [128 edges, chunk, 128 feat]; a one-hot
           scatter matrix S (built on DVE) is the matmul stationary so PSUM
           accumulates psum[dst, feat] += S^T @ M exactly in fp32.  A bias
           matmul initializes each window; ACT applies relu; batched DMA
           writes the output (bf16, cast to f32 on host).

hw rows are stored partition-major per core shard:
  storage_row(node r) = (r % 128) * NWIN + r // 128
so no on-device transposes are needed; host index math absorbs it.
"""

import numpy as np
import ml_dtypes
from contextlib import ExitStack

import concourse.bacc as bacc
import concourse.bass as bass
import concourse.mybir as mybir
import concourse.tile as tile
from concourse.bass_utils import run_bass_kernel_spmd

BF16 = mybir.dt.bfloat16
F32 = mybir.dt.float32
I16 = mybir.dt.int16
NPBF16 = ml_dtypes.bfloat16

NC = 8
SB = 16  # one-hot lanes per DVE build op (j-major layout)


def cdiv(a, b):
    return (a + b - 1) // b


class _Meta:
    def __init__(self, n_nodes, in_feats, out_feats, wgroup, chunks):
        assert n_nodes % NC == 0
        self.N = n_nodes
        self.IN = in_feats
        self.OUT = out_feats
        self.SHARD = n_nodes // NC
        self.SHARD_PAD = cdiv(self.SHARD, 128) * 128
        self.NWIN = self.SHARD_PAD // 128
        self.HALFPAD = (NC // 2) * self.SHARD_PAD
        self.KCH = in_feats // 128
        self.WGROUP = wgroup
        self.chunks = chunks
        self.groups = [
            list(range(g, min(g + wgroup, self.NWIN)))
            for g in range(0, self.NWIN, wgroup)
        ]
        self.calls = []
        ch = 0
        for gi, g in enumerate(self.groups):
            for hh in (0, 1):
                nch_call = int(sum(self.chunks[w][hh] for w in g))
                if nch_call == 0:
                    continue
                self.calls.append((gi, hh, nch_call, ch))
                ch += nch_call
        self.NCH = ch
        self.GTOT = ch * 128

    def chunk_base(self, gi, hh, w):
        base = 0
        for (g2, h2, nch, cb) in self.calls:
            if g2 == gi and h2 == hh:
                for w2 in self.groups[gi]:
                    if w2 == w:
                        return base + cb
                    base += int(self.chunks[w2][h2])
        return None


def _prepare(h, W, b, src, dst, wgroup=8):
    n_nodes, in_feats = h.shape
    out_feats = W.shape[1]

    src = np.asarray(src, dtype=np.int64)
    dst = np.asarray(dst, dtype=np.int64)
    SHARD = n_nodes // NC
    SHARD_PAD = cdiv(SHARD, 128) * 128
    NWIN = SHARD_PAD // 128
    HALFPAD = (NC // 2) * SHARD_PAD
    assert HALFPAD <= 32768

    core = dst // SHARD
    rloc = dst - core * SHARD
    w_of = rloc // 128
    slot = rloc % 128
    prow = (src // SHARD) * SHARD_PAD + ((src % SHARD) % 128) * NWIN + (
        src % SHARD
    ) // 128
    half = (prow >= HALFPAD).astype(np.int64)

    counts = np.zeros((NC, NWIN, 2), np.int64)
    np.add.at(counts, (core, w_of, half), 1)
    chunks = np.ceil(counts.max(axis=0) / 128).astype(int)

    meta = _Meta(n_nodes, in_feats, out_feats, wgroup, chunks)

    W_bf = np.ascontiguousarray(W.astype(NPBF16))
    brep = np.ascontiguousarray(np.tile(b.astype(NPBF16)[None, :], (128, 1)))
    ident = np.eye(128, dtype=NPBF16)
    iota = np.repeat(np.arange(128, dtype=NPBF16), SB)[None, :]
    iota = np.ascontiguousarray(np.tile(iota, (128, 1)))

    in_maps = []
    order = np.lexsort((w_of, half, core))
    so_core, so_half, so_w = core[order], half[order], w_of[order]
    so_prow, so_slot = prow[order], slot[order]
    keys = (so_core * 2 + so_half) * NWIN + so_w
    uniq, starts = np.unique(keys, return_index=True)
    starts = list(starts) + [len(keys)]
    seg = {int(k): (int(s), int(e)) for k, s, e in zip(uniq, starts[:-1], starts[1:])}

    for c in range(NC):
        idx_stream = np.zeros(meta.GTOT, np.int16)
        dstw_stream = np.full(meta.GTOT, 255.0, np.float32)
        pos = 0
        for (gi, hh, nch_call, cb) in meta.calls:
            for w in meta.groups[gi]:
                ntok = int(meta.chunks[w][hh]) * 128
                if ntok == 0:
                    continue
                k = (c * 2 + hh) * NWIN + w
                if k in seg:
                    s, e = seg[k]
                    kk = e - s
                    idx_stream[pos : pos + kk] = (
                        so_prow[s:e] - hh * HALFPAD
                    ).astype(np.int16)
                    dstw_stream[pos : pos + kk] = so_slot[s:e]
                pos += ntok

        gidx = np.zeros((16, meta.GTOT // 16), np.int16)
        for (gi, hh, nch_call, cb) in meta.calls:
            t0, ntok = cb * 128, nch_call * 128
            seg16 = idx_stream[t0 : t0 + ntok].reshape(ntok // 16, 16).T
            gidx[:, t0 // 16 : (t0 + ntok) // 16] = seg16
        gidx = np.ascontiguousarray(np.tile(gidx, (8, 1)))

        dstw = np.ascontiguousarray(
            dstw_stream.reshape(meta.NCH, 128).T.astype(NPBF16)
        )

        hT = np.zeros((in_feats, SHARD_PAD), NPBF16)
        hT[:, :SHARD] = h[c * SHARD : (c + 1) * SHARD].T.astype(NPBF16)

        in_maps.append(
            {
                "hT": hT,
                "Wt": W_bf,
                "brep": brep,
                "ident": ident,
                "iotarep": iota,
                "gidx": gidx,
                "dstw": dstw,
            }
        )

    def unpermute(outs):
        res = np.empty((n_nodes, out_feats), np.float32)
        for c in range(NC):
            arr = np.asarray(outs[c]["out"], dtype=np.float32)
            rows = arr.transpose(1, 0, 2).reshape(SHARD_PAD, out_feats)
            res[c * SHARD : (c + 1) * SHARD] = rows[:SHARD]
        return res

    return meta, in_maps, unpermute


def _build_kernel(meta):
    m = meta
    nc = bacc.Bacc("TRN2", target_bir_lowering=False, num_devices=NC)

    hT = nc.dram_tensor("hT", [m.IN, m.SHARD_PAD], BF16, kind="ExternalInput")
    Wt = nc.dram_tensor("Wt", [m.IN, m.OUT], BF16, kind="ExternalInput")
    brep = nc.dram_tensor("brep", [128, m.OUT], BF16, kind="ExternalInput")
    ident = nc.dram_tensor("ident", [128, 128], BF16, kind="ExternalInput")
    iotarep = nc.dram_tensor(
        "iotarep", [128, 128 * SB], BF16, kind="ExternalInput"
    )
    gidx = nc.dram_tensor("gidx", [128, m.GTOT // 16], I16, kind="ExternalInput")
    dstw = nc.dram_tensor("dstw", [128, m.NCH], BF16, kind="ExternalInput")
    out = nc.dram_tensor("out", [128, m.NWIN, m.OUT], BF16, kind="ExternalOutput")

    maxch_call = max(c[2] for c in m.calls)

    with tile.TileContext(nc, num_cores=NC) as tc, ExitStack() as ctx:
        consts = ctx.enter_context(tc.tile_pool(name="consts", bufs=1))
        dram = ctx.enter_context(tc.tile_pool(name="dram", bufs=1, space="DRAM"))
        psum_pool = ctx.enter_context(
            tc.tile_pool(name="psum", bufs=4, space="PSUM")
        )
        sbuf = ctx.enter_context(tc.tile_pool(name="sbuf", bufs=2))
        spool = ctx.enter_context(tc.tile_pool(name="spool", bufs=6))

        wt_sb = consts.tile([128, m.KCH, m.OUT], BF16)
        for k in range(m.KCH):
            nc.sync.dma_start(wt_sb[:, k, :], Wt[k * 128 : (k + 1) * 128, :])
        brep_sb = consts.tile([128, m.OUT], BF16)
        nc.sync.dma_start(brep_sb[:], brep[:])
        ident_sb = consts.tile([128, 128], BF16)
        nc.sync.dma_start(ident_sb[:], ident[:])
        iota_sb = consts.tile([128, 128 * SB], BF16)
        nc.sync.dma_start(iota_sb[:], iotarep[:])
        gidx_sb = consts.tile([128, m.GTOT // 16], I16)
        nc.sync.dma_start(gidx_sb[:], gidx[:])
        dstw_sb = consts.tile([128, m.NCH], BF16)
        nc.sync.dma_start(dstw_sb[:], dstw[:])
        hT_sb = consts.tile([128, m.KCH, m.SHARD_PAD], BF16)
        for k in range(m.KCH):
            nc.sync.dma_start(hT_sb[:, k, :], hT[k * 128 : (k + 1) * 128, :])

        hw_shard = dram.tile([128, m.NWIN, m.OUT], BF16)
        hw_full = dram.tile(
            [NC * m.SHARD_PAD, m.OUT], BF16, addr_space="Shared"
        )

        hw_sb = consts.tile([128, m.NWIN, m.OUT], BF16)
        for blk in range(m.NWIN):
            ps = psum_pool.tile([128, m.OUT], F32, tag="p1")
            for k in range(m.KCH):
                nc.tensor.matmul(
                    ps[:],
                    lhsT=hT_sb[:, k, blk * 128 : (blk + 1) * 128],
                    rhs=wt_sb[:, k, :],
                    start=(k == 0),
                    stop=(k == m.KCH - 1),
                )
            nc.vector.tensor_copy(hw_sb[:, blk, :], ps[:])
        nc.sync.dma_start(hw_shard[:], hw_sb[:])
        nc.gpsimd.collective_compute(
            "AllGather",
            mybir.AluOpType.bypass,
            replica_groups=[list(range(NC))],
            ins=[hw_shard.opt()],
            outs=[hw_full.opt()],
        )

        call_of = {}
        for (gi, hh, nch_call, cb) in m.calls:
            call_of[(gi, hh)] = (None, nch_call, cb)

        for gi, g in enumerate(m.groups):
            gtiles = {}
            for hh in (0, 1):
                if (gi, hh) not in call_of:
                    continue
                _, nch_call, cb = call_of[(gi, hh)]
                t = sbuf.tile([128, maxch_call, m.OUT], BF16, tag=f"gt{hh}")
                nc.gpsimd.dma_gather(
                    t[:, :nch_call, :],
                    hw_full[hh * m.HALFPAD : (hh + 1) * m.HALFPAD, :],
                    gidx_sb[:, cb * 8 : (cb + nch_call) * 8],
                    nch_call * 128,
                    nch_call * 128,
                    m.OUT,
                    single_packet=False,
                )
                gtiles[hh] = (t, cb)

            ot = sbuf.tile([128, len(g), m.OUT], BF16, tag="ot")
            for wi, w in enumerate(g):
                wchunks = []
                for hh in (0, 1):
                    nch_w = int(m.chunks[w][hh])
                    if nch_w == 0:
                        continue
                    cb0 = m.chunk_base(gi, hh, w)
                    _, _, cb_call = call_of[(gi, hh)]
                    for q in range(nch_w):
                        wchunks.append((cb0 + q, hh, cb0 + q - cb_call))

                ps = psum_pool.tile([128, m.OUT], F32, tag="p2")
                nc.tensor.matmul(
                    ps[:],
                    lhsT=ident_sb[:],
                    rhs=brep_sb[:],
                    start=True,
                    stop=(len(wchunks) == 0),
                )
                smap = {}
                sg = 0
                while sg < len(wchunks):
                    ch0 = wchunks[sg][0]
                    bsz = 1
                    while (
                        bsz < SB
                        and sg + bsz < len(wchunks)
                        and wchunks[sg + bsz][0] == ch0 + bsz
                    ):
                        bsz += 1
                    st = spool.tile([128, 128 * SB], BF16, tag="S")
                    stv = st[:].rearrange("p (j c) -> p j c", c=SB)
                    iov = iota_sb[:].rearrange("p (j c) -> p j c", c=SB)
                    if bsz >= 2:
                        in0 = (
                            dstw_sb[:, ch0 : ch0 + bsz]
                            .unsqueeze(1)
                            .broadcast_to([128, 128, bsz])
                        )
                    else:
                        in0 = (
                            dstw_sb[:, ch0 : ch0 + 1]
                            .unsqueeze(1)
                            .broadcast_to([128, 128, 1])
                        )
                    nc.vector.tensor_tensor(
                        out=stv[:, :, :bsz],
                        in0=in0,
                        in1=iov[:, :, :bsz],
                        op=mybir.AluOpType.is_equal,
                    )
                    for i in range(bsz):
                        smap[sg + i] = (stv, i)
                    sg += bsz
                for qi, (gch, hh, lch) in enumerate(wchunks):
                    stv, lane = smap[qi]
                    gt, _ = gtiles[hh]
                    nc.tensor.matmul(
                        ps[:],
                        lhsT=stv[:, :, lane],
                        rhs=gt[:, lch, :],
                        start=False,
                        stop=(qi == len(wchunks) - 1),
                    )
                nc.scalar.activation(
                    ot[:, wi, :], ps[:], mybir.ActivationFunctionType.Relu
                )
            nc.sync.dma_start(out[:, g[0] : g[0] + len(g), :], ot[:])

    nc.compile()
    return nc


def kernel(h, W, b, src, dst):
    h = np.asarray(h, dtype=np.float32)
    W = np.asarray(W, dtype=np.float32)
    b = np.asarray(b, dtype=np.float32)

    meta, in_maps, unpermute = _prepare(h, W, b, src, dst, wgroup=8)
    nc = _build_kernel(meta)
    res = run_bass_kernel_spmd(nc, in_maps, core_ids=list(range(NC)))
    return unpermute(res.results)


# revision 4
# speedup vs baseline: 1931.7841x; 1.3603x over previous
"""GCN layer on 8 TRN2 NeuronCores (Bass/Tile kernel).

out = relu(segment_sum((h @ W)[src], dst) + b)

Self-contained: hardcodes the problem shapes (N=50000, IN=256, OUT=128,
E=800000) and the sharding strategy.

Strategy (dst-sharded, 8 cores):
  Phase 1: each core computes hw = h_shard @ W in bf16 on the PE, writes its
           shard to internal DRAM, AllGather -> full hw table on every core.
  Phase 2: edges partitioned by dst owner; per core, edges grouped by
           (dst window of 128 rows, src half table) and padded to 128-edge
           chunks.  dma_gather (SWDGE) pulls per-edge message rows hw[src]
           from DRAM into SBUF [128 edges, chunk, 128 feat]; a one-hot
           scatter matrix S (built on DVE) is the matmul stationary so PSUM
           accumulates psum[dst, feat] += S^T @ M exactly in fp32.  A bias
           matmul initializes each window; ACT applies relu; batched DMA
           writes the output (bf16, cast to f32 on host).

hw rows are stored partition-major per core shard:
  storage_row(node r) = (r % 128) * NWIN + r // 128
so no on-device transposes are needed; host index math absorbs it.
"""

import numpy as np
import ml_dtypes
from contextlib import ExitStack

import concourse.bacc as bacc
import concourse.bass as bass
import concourse.mybir as mybir
import concourse.tile as tile
from concourse.bass_utils import run_bass_kernel_spmd

BF16 = mybir.dt.bfloat16
F32 = mybir.dt.float32
I16 = mybir.dt.int16
NPBF16 = ml_dtypes.bfloat16

NC = 8
SB = 16  # one-hot lanes per DVE build op (j-major layout)


def cdiv(a, b):
    return (a + b - 1) // b


class _Meta:
    def __init__(self, n_nodes, in_feats, out_feats, wgroup, chunks):
        assert n_nodes % NC == 0
        self.N = n_nodes
        self.IN = in_feats
        self.OUT = out_feats
        self.SHARD = n_nodes // NC
        self.SHARD_PAD = cdiv(self.SHARD, 128) * 128
        self.NWIN = self.SHARD_PAD // 128
        self.HALFPAD = (NC // 2) * self.SHARD_PAD
        self.KCH = in_feats // 128
        self.WGROUP = wgroup
        self.chunks = chunks
        self.groups = [
            list(range(g, min(g + wgroup, self.NWIN)))
            for g in range(0, self.NWIN, wgroup)
        ]
        self.calls = []
        ch = 0
        for gi, g in enumerate(self.groups):
            for hh in (0, 1):
                nch_call = int(sum(self.chunks[w][hh] for w in g))
                if nch_call == 0:
                    continue
                self.calls.append((gi, hh, nch_call, ch))
                ch += nch_call
        self.NCH = ch
        self.GTOT = ch * 128

    def chunk_base(self, gi, hh, w):
        base = 0
        for (g2, h2, nch, cb) in self.calls:
            if g2 == gi and h2 == hh:
                for w2 in self.groups[gi]:
                    if w2 == w:
                        return base + cb
                    base += int(self.chunks[w2][h2])
        return None


def _prepare(h, W, b, src, dst, wgroup=8):
    n_nodes, in_feats = h.shape
    out_feats = W.shape[1]

    src = np.asarray(src, dtype=np.int64)
    dst = np.asarray(dst, dtype=np.int64)
    SHARD = n_nodes // NC
    SHARD_PAD = cdiv(SHARD, 128) * 128
    NWIN = SHARD_PAD // 128
    HALFPAD = (NC // 2) * SHARD_PAD
    assert HALFPAD <= 32768

    core = dst // SHARD
    rloc = dst - core * SHARD
    w_of = rloc // 128
    slot = rloc % 128
    prow = (src // SHARD) * SHARD_PAD + ((src % SHARD) % 128) * NWIN + (
        src % SHARD
    ) // 128
    half = (prow >= HALFPAD).astype(np.int64)

    counts = np.zeros((NC, NWIN, 2), np.int64)
    np.add.at(counts, (core, w_of, half), 1)
    chunks = np.ceil(counts.max(axis=0) / 128).astype(int)

    meta = _Meta(n_nodes, in_feats, out_feats, wgroup, chunks)

    W_bf = np.ascontiguousarray(W.astype(NPBF16))
    brep = np.ascontiguousarray(np.tile(b.astype(NPBF16)[None, :], (128, 1)))
    ident = np.eye(128, dtype=NPBF16)
    iota = np.repeat(np.arange(128, dtype=NPBF16), SB)[None, :]
    iota = np.ascontiguousarray(np.tile(iota, (128, 1)))

    in_maps = []
    order = np.lexsort((w_of, half, core))
    so_core, so_half, so_w = core[order], half[order], w_of[order]
    so_prow, so_slot = prow[order], slot[order]
    keys = (so_core * 2 + so_half) * NWIN + so_w
    uniq, starts = np.unique(keys, return_index=True)
    starts = list(starts) + [len(keys)]
    seg = {int(k): (int(s), int(e)) for k, s, e in zip(uniq, starts[:-1], starts[1:])}

    for c in range(NC):
        idx_stream = np.zeros(meta.GTOT, np.int16)
        dstw_stream = np.full(meta.GTOT, 255.0, np.float32)
        pos = 0
        for (gi, hh, nch_call, cb) in meta.calls:
            for w in meta.groups[gi]:
                ntok = int(meta.chunks[w][hh]) * 128
                if ntok == 0:
                    continue
                k = (c * 2 + hh) * NWIN + w
                if k in seg:
                    s, e = seg[k]
                    kk = e - s
                    idx_stream[pos : pos + kk] = (
                        so_prow[s:e] - hh * HALFPAD
                    ).astype(np.int16)
                    dstw_stream[pos : pos + kk] = so_slot[s:e]
                pos += ntok

        gidx = np.zeros((16, meta.GTOT // 16), np.int16)
        for (gi, hh, nch_call, cb) in meta.calls:
            t0, ntok = cb * 128, nch_call * 128
            seg16 = idx_stream[t0 : t0 + ntok].reshape(ntok // 16, 16).T
            gidx[:, t0 // 16 : (t0 + ntok) // 16] = seg16
        gidx = np.ascontiguousarray(np.tile(gidx, (8, 1)))

        dstw = np.ascontiguousarray(
            dstw_stream.reshape(meta.NCH, 128).T.astype(NPBF16)
        )

        hT = np.zeros((in_feats, SHARD_PAD), NPBF16)
        hT[:, :SHARD] = h[c * SHARD : (c + 1) * SHARD].T.astype(NPBF16)

        in_maps.append(
            {
                "hT": hT,
                "Wt": W_bf,
                "brep": brep,
                "ident": ident,
                "iotarep": iota,
                "gidx": gidx,
                "dstw": dstw,
            }
        )

    def unpermute(outs):
        res = np.empty((n_nodes, out_feats), np.float32)
        for c in range(NC):
            arr = np.asarray(outs[c]["out"], dtype=np.float32)
            rows = arr.transpose(1, 0, 2).reshape(SHARD_PAD, out_feats)
            res[c * SHARD : (c + 1) * SHARD] = rows[:SHARD]
        return res

    return meta, in_maps, unpermute


def _build_kernel(meta):
    m = meta
    nc = bacc.Bacc("TRN2", target_bir_lowering=False, num_devices=NC)

    hT = nc.dram_tensor("hT", [m.IN, m.SHARD_PAD], BF16, kind="ExternalInput")
    Wt = nc.dram_tensor("Wt", [m.IN, m.OUT], BF16, kind="ExternalInput")
    brep = nc.dram_tensor("brep", [128, m.OUT], BF16, kind="ExternalInput")
    ident = nc.dram_tensor("ident", [128, 128], BF16, kind="ExternalInput")
    iotarep = nc.dram_tensor(
        "iotarep", [128, 128 * SB], BF16, kind="ExternalInput"
    )
    gidx = nc.dram_tensor("gidx", [128, m.GTOT // 16], I16, kind="ExternalInput")
    dstw = nc.dram_tensor("dstw", [128, m.NCH], BF16, kind="ExternalInput")
    out = nc.dram_tensor("out", [128, m.NWIN, m.OUT], BF16, kind="ExternalOutput")

    maxch_call = max(c[2] for c in m.calls)

    with tile.TileContext(nc, num_cores=NC) as tc, ExitStack() as ctx:
        consts = ctx.enter_context(tc.tile_pool(name="consts", bufs=1))
        dram = ctx.enter_context(tc.tile_pool(name="dram", bufs=1, space="DRAM"))
        psum_pool = ctx.enter_context(
            tc.tile_pool(name="psum", bufs=4, space="PSUM")
        )
        sbuf = ctx.enter_context(tc.tile_pool(name="sbuf", bufs=2))
        gpool = ctx.enter_context(tc.tile_pool(name="gpool", bufs=3))
        spool = ctx.enter_context(tc.tile_pool(name="spool", bufs=10))

        wt_sb = consts.tile([128, m.KCH, m.OUT], BF16)
        for k in range(m.KCH):
            nc.sync.dma_start(wt_sb[:, k, :], Wt[k * 128 : (k + 1) * 128, :])
        brep_sb = consts.tile([128, m.OUT], BF16)
        nc.sync.dma_start(brep_sb[:], brep[:])
        ident_sb = consts.tile([128, 128], BF16)
        nc.sync.dma_start(ident_sb[:], ident[:])
        iota_sb = consts.tile([128, 128 * SB], BF16)
        nc.sync.dma_start(iota_sb[:], iotarep[:])
        gidx_sb = consts.tile([128, m.GTOT // 16], I16)
        nc.sync.dma_start(gidx_sb[:], gidx[:])
        dstw_sb = consts.tile([128, m.NCH], BF16)
        nc.sync.dma_start(dstw_sb[:], dstw[:])
        hT_sb = consts.tile([128, m.KCH, m.SHARD_PAD], BF16)
        for k in range(m.KCH):
            nc.sync.dma_start(hT_sb[:, k, :], hT[k * 128 : (k + 1) * 128, :])

        hw_shard = dram.tile([128, m.NWIN, m.OUT], BF16)
        hw_full = dram.tile(
            [NC * m.SHARD_PAD, m.OUT], BF16, addr_space="Shared"
        )

        hw_sb = consts.tile([128, m.NWIN, m.OUT], BF16)
        for blk in range(m.NWIN):
            ps = psum_pool.tile([128, m.OUT], F32, tag="p1")
            for k in range(m.KCH):
                nc.tensor.matmul(
                    ps[:],
                    lhsT=hT_sb[:, k, blk * 128 : (blk + 1) * 128],
                    rhs=wt_sb[:, k, :],
                    start=(k == 0),
                    stop=(k == m.KCH - 1),
                )
            nc.vector.tensor_copy(hw_sb[:, blk, :], ps[:])
        nc.sync.dma_start(hw_shard[:], hw_sb[:])
        nc.gpsimd.collective_compute(
            "AllGather",
            mybir.AluOpType.bypass,
            replica_groups=[list(range(NC))],
            ins=[hw_shard.opt()],
            outs=[hw_full.opt()],
        )

        call_of = {}
        for (gi, hh, nch_call, cb) in m.calls:
            call_of[(gi, hh)] = (None, nch_call, cb)

        for gi, g in enumerate(m.groups):
            gtiles = {}
            for hh in (0, 1):
                if (gi, hh) not in call_of:
                    continue
                _, nch_call, cb = call_of[(gi, hh)]
                t = gpool.tile([128, maxch_call, m.OUT], BF16, tag=f"gt{hh}")
                nc.gpsimd.dma_gather(
                    t[:, :nch_call, :],
                    hw_full[hh * m.HALFPAD : (hh + 1) * m.HALFPAD, :],
                    gidx_sb[:, cb * 8 : (cb + nch_call) * 8],
                    nch_call * 128,
                    nch_call * 128,
                    m.OUT,
                    single_packet=False,
                )
                gtiles[hh] = (t, cb)

            ot = sbuf.tile([128, len(g), m.OUT], BF16, tag="ot")
            for wi, w in enumerate(g):
                wchunks = []
                for hh in (0, 1):
                    nch_w = int(m.chunks[w][hh])
                    if nch_w == 0:
                        continue
                    cb0 = m.chunk_base(gi, hh, w)
                    _, _, cb_call = call_of[(gi, hh)]
                    for q in range(nch_w):
                        wchunks.append((cb0 + q, hh, cb0 + q - cb_call))

                ps = psum_pool.tile([128, m.OUT], F32, tag="p2")
                nc.tensor.matmul(
                    ps[:],
                    lhsT=ident_sb[:],
                    rhs=brep_sb[:],
                    start=True,
                    stop=(len(wchunks) == 0),
                )
                smap = {}
                sg = 0
                while sg < len(wchunks):
                    ch0 = wchunks[sg][0]
                    bsz = 1
                    while (
                        bsz < SB
                        and sg + bsz < len(wchunks)
                        and wchunks[sg + bsz][0] == ch0 + bsz
                    ):
                        bsz += 1
                    st = spool.tile([128, 128 * SB], BF16, tag="S")
                    stv = st[:].rearrange("p (j c) -> p j c", c=SB)
                    iov = iota_sb[:].rearrange("p (j c) -> p j c", c=SB)
                    if bsz >= 2:
                        in0 = (
                            dstw_sb[:, ch0 : ch0 + bsz]
                            .unsqueeze(1)
                            .broadcast_to([128, 128, bsz])
                        )
                    else:
                        in0 = (
                            dstw_sb[:, ch0 : ch0 + 1]
                            .unsqueeze(1)
                            .broadcast_to([128, 128, 1])
                        )
                    nc.vector.tensor_tensor(
                        out=stv[:, :, :bsz],
                        in0=in0,
                        in1=iov[:, :, :bsz],
                        op=mybir.AluOpType.is_equal,
                    )
                    for i in range(bsz):
                        smap[sg + i] = (stv, i)
                    sg += bsz
                for qi, (gch, hh, lch) in enumerate(wchunks):
                    stv, lane = smap[qi]
                    gt, _ = gtiles[hh]
                    nc.tensor.matmul(
                        ps[:],
                        lhsT=stv[:, :, lane],
                        rhs=gt[:, lch, :],
                        start=False,
                        stop=(qi == len(wchunks) - 1),
                    )
                nc.scalar.activation(
                    ot[:, wi, :], ps[:], mybir.ActivationFunctionType.Relu
                )
            nc.sync.dma_start(out[:, g[0] : g[0] + len(g), :], ot[:])

    nc.compile()
    return nc


def kernel(h, W, b, src, dst):
    h = np.asarray(h, dtype=np.float32)
    W = np.asarray(W, dtype=np.float32)
    b = np.asarray(b, dtype=np.float32)

    meta, in_maps, unpermute = _prepare(h, W, b, src, dst, wgroup=4)
    nc = _build_kernel(meta)
    res = run_bass_kernel_spmd(nc, in_maps, core_ids=list(range(NC)))
    return unpermute(res.results)
